# revision 1
# baseline (speedup 1.0000x reference)
"""Trainium2 Bass kernel for nn_NodeNetwork (GNN message passing + MLP + L2 norm).

Data-parallel over the node dimension: 500000 nodes sharded as 62500/core
across 8 NeuronCores; MLP weights replicated. Per core, nodes are processed
in 125 macro-tiles of 500 nodes (4 groups x 125 partitions).
"""

import numpy as np

F = 128
DEG = 16
H1 = 256
H2 = 256
OUT = 128
N_CORES = 8

G = 125          # nodes per partition-group
NG = 4           # groups per macro-tile
MACRO = G * NG   # 500 nodes per macro-tile

_NC_CACHE = {}


def build(n_nodes, n_cores=N_CORES, mode="full"):
    import concourse.bacc as bacc
    import concourse.mybir as mybir
    import concourse.tile as tile
    import concourse.masks as masks
    from contextlib import ExitStack

    f32 = mybir.dt.float32
    AX = mybir.AxisListType
    ALU = mybir.AluOpType
    ACTF = mybir.ActivationFunctionType

    assert n_nodes % MACRO == 0
    n_macros = n_nodes // MACRO

    nc = bacc.Bacc(
        "TRN2", target_bir_lowering=False, debug=False, num_devices=n_cores
    )
    msg_d = nc.dram_tensor("message", [n_nodes, DEG * F], f32, kind="ExternalInput").ap()
    feat_d = nc.dram_tensor("features", [n_nodes, F], f32, kind="ExternalInput").ap()
    glob_d = nc.dram_tensor(
        "global_features", [n_nodes, F], f32, kind="ExternalInput"
    ).ap()
    w1_d = nc.dram_tensor("W1", [3 * F, H1], f32, kind="ExternalInput").ap()
    b1_d = nc.dram_tensor("b1", [H1], f32, kind="ExternalInput").ap()
    w2_d = nc.dram_tensor("W2", [H1, H2], f32, kind="ExternalInput").ap()
    b2_d = nc.dram_tensor("b2", [H2], f32, kind="ExternalInput").ap()
    w3_d = nc.dram_tensor("W3", [H2, OUT], f32, kind="ExternalInput").ap()
    b3_d = nc.dram_tensor("b3", [OUT], f32, kind="ExternalInput").ap()
    out_d = nc.dram_tensor("out", [n_nodes, OUT], f32, kind="ExternalOutput").ap()

    with tile.TileContext(nc) as tc, ExitStack() as ctx:
        cpool = ctx.enter_context(tc.tile_pool(name="consts", bufs=1))
        mpool = ctx.enter_context(tc.tile_pool(name="msg", bufs=3))
        ipool = ctx.enter_context(tc.tile_pool(name="inputs", bufs=3))
        xpool = ctx.enter_context(tc.tile_pool(name="xside", bufs=3))
        xtpool = ctx.enter_context(tc.tile_pool(name="xt", bufs=4))
        hpool = ctx.enter_context(tc.tile_pool(name="hid", bufs=2))
        npool = ctx.enter_context(tc.tile_pool(name="norm", bufs=2))
        opool = ctx.enter_context(tc.tile_pool(name="outp", bufs=3))
        ps_xt = ctx.enter_context(tc.tile_pool(name="ps_xt", bufs=2, space="PSUM"))
        ps_mm = ctx.enter_context(tc.tile_pool(name="ps_mm", bufs=3, space="PSUM"))
        ps_out = ctx.enter_context(tc.tile_pool(name="ps_out", bufs=2, space="PSUM"))

        # --- constants ---
        ident = cpool.tile([128, 128], f32, tag="ident")
        masks.make_identity(nc, ident[:])
        w1sb = cpool.tile([128, 3 * H1], f32, tag="w1")  # [p, (k=3, m*128+c=256)]
        nc.sync.dma_start(w1sb[:], w1_d.rearrange("(k p) m -> p k m", p=128))
        w2sb = cpool.tile([128, 2 * H2], f32, tag="w2")
        nc.sync.dma_start(w2sb[:], w2_d.rearrange("(k p) m -> p k m", p=128))
        w3sb = cpool.tile([128, 2 * OUT], f32, tag="w3")
        nc.sync.dma_start(w3sb[:], w3_d.rearrange("(k p) m -> p k m", p=128))
        b1sb = cpool.tile([128, 2], f32, tag="b1")
        nc.sync.dma_start(b1sb[:], b1_d.rearrange("(m p) -> p m", p=128))
        b2sb = cpool.tile([128, 2], f32, tag="b2")
        nc.sync.dma_start(b2sb[:], b2_d.rearrange("(m p) -> p m", p=128))
        b3sb = cpool.tile([128, 1], f32, tag="b3")
        nc.sync.dma_start(b3sb[:], b3_d.rearrange("(m p) -> p m", p=128))

        if mode == "noin":
            z = opool.tile([128, NG * F], f32, tag="outsb")
            nc.gpsimd.memset(z[:], 0.25)
            for mi in range(n_macros):
                r0 = mi * MACRO
                nc.scalar.dma_start(
                    out_d[r0 : r0 + MACRO].rearrange("(g p) f -> p g f", p=G),
                    z[:G].rearrange("p (g f) -> p g f", g=NG),
                )
            n_macros = 0

        for mi in range(n_macros):
            r0 = mi * MACRO
            # --- loads ---
            msgt = mpool.tile([128, NG * DEG * F], f32, tag="msgt")
            if mode == "dma_h":
                nc.sync.dma_start(
                    msgt[:G].rearrange("p (g j) -> p g j", g=NG)[:, :, : DEG * F // 2],
                    msg_d[r0 : r0 + MACRO]
                    .rearrange("(g p) j -> p g j", p=G)[:, :, : DEG * F // 2],
                )
            elif mode == "dma_gp":
                nc.gpsimd.dma_start(
                    msgt[:G].rearrange("p (g j) -> p g j", g=NG),
                    msg_d[r0 : r0 + MACRO].rearrange("(g p) j -> p g j", p=G),
                )
            else:
                nc.sync.dma_start(
                    msgt[:G].rearrange("p (g j) -> p g j", g=NG),
                    msg_d[r0 : r0 + MACRO].rearrange("(g p) j -> p g j", p=G),
                )
            featt = ipool.tile([128, NG * F], f32, tag="featt")
            nc.sync.dma_start(
                featt[:G].rearrange("p (g f) -> p g f", g=NG),
                feat_d[r0 : r0 + MACRO].rearrange("(g p) f -> p g f", p=G),
            )
            globt = ipool.tile([128, NG * F], f32, tag="globt")
            nc.sync.dma_start(
                globt[:G].rearrange("p (g f) -> p g f", g=NG),
                glob_d[r0 : r0 + MACRO].rearrange("(g p) f -> p g f", p=G),
            )

            if mode != "full":
                xagg = xpool.tile([128, NG * F], f32, tag="xagg")
                if mode == "dma_agg":
                    for g in range(NG):
                        nc.vector.tensor_reduce(
                            xagg[:G, g * F : (g + 1) * F],
                            msgt[:G, g * DEG * F : (g + 1) * DEG * F].rearrange(
                                "p (d f) -> p f d", f=F
                            ),
                            axis=AX.X,
                            op=ALU.add,
                        )
                else:
                    nc.vector.tensor_copy(xagg[:G], msgt[:G, : NG * F])
                outsb = opool.tile([128, NG * F], f32, tag="outsb")
                nc.vector.tensor_add(outsb[:G], xagg[:G], featt[:G])
                nc.scalar.dma_start(
                    out_d[r0 : r0 + MACRO].rearrange("(g p) f -> p g f", p=G),
                    outsb[:G].rearrange("p (g f) -> p g f", g=NG),
                )
                continue

            # --- mailbox sum over DEG (node-major) ---
            xagg = xpool.tile([128, NG * F], f32, tag="xagg")
            for g in range(NG):
                nc.vector.tensor_reduce(
                    xagg[:G, g * F : (g + 1) * F],
                    msgt[:G, g * DEG * F : (g + 1) * DEG * F].rearrange(
                        "p (d f) -> p f d", f=F
                    ),
                    axis=AX.X,
                    op=ALU.add,
                )

            # --- transpose x pieces to feature-major [128, 500] ---
            xts = []
            for src in (xagg, featt, globt):
                pxt = ps_xt.tile([128, 512], f32, tag="pxt")
                for g in range(NG):
                    nc.tensor.transpose(
                        pxt[:, g * G : (g + 1) * G],
                        src[:G, g * F : (g + 1) * F],
                        ident[:G, :G],
                    )
                xt = xtpool.tile([128, MACRO], f32, tag="xt")
                nc.scalar.copy(xt[:], pxt[:, :MACRO])
                xts.append(xt)

            # --- layer 1: [384 -> 256], relu ---
            h1 = hpool.tile([128, 2 * MACRO], f32, tag="h1")
            for m in range(2):
                pmm = ps_mm.tile([128, MACRO], f32, tag="pmm")
                for k in range(3):
                    nc.tensor.matmul(
                        pmm[:],
                        w1sb[:, k * H1 + m * 128 : k * H1 + (m + 1) * 128],
                        xts[k][:],
                        start=(k == 0),
                        stop=(k == 2),
                    )
                nc.scalar.activation(
                    h1[:, m * MACRO : (m + 1) * MACRO],
                    pmm[:],
                    ACTF.Relu,
                    bias=b1sb[:, m : m + 1],
                )

            # --- layer 2: [256 -> 256], relu ---
            h2 = hpool.tile([128, 2 * MACRO], f32, tag="h2")
            for m in range(2):
                pmm = ps_mm.tile([128, MACRO], f32, tag="pmm")
                for k in range(2):
                    nc.tensor.matmul(
                        pmm[:],
                        w2sb[:, k * H2 + m * 128 : k * H2 + (m + 1) * 128],
                        h1[:, k * MACRO : (k + 1) * MACRO],
                        start=(k == 0),
                        stop=(k == 1),
                    )
                nc.scalar.activation(
                    h2[:, m * MACRO : (m + 1) * MACRO],
                    pmm[:],
                    ACTF.Relu,
                    bias=b2sb[:, m : m + 1],
                )

            # --- layer 3: [256 -> 128], + b3 ---
            pmm = ps_mm.tile([128, MACRO], f32, tag="pmm")
            for k in range(2):
                nc.tensor.matmul(
                    pmm[:],
                    w3sb[:, k * OUT : (k + 1) * OUT],
                    h2[:, k * MACRO : (k + 1) * MACRO],
                    start=(k == 0),
                    stop=(k == 1),
                )
            o3 = hpool.tile([128, MACRO], f32, tag="o3")
            nc.scalar.activation(o3[:], pmm[:], ACTF.Identity, bias=b3sb[:, 0:1])

            # --- transpose back to node-major ---
            pout = ps_out.tile([128, NG * F], f32, tag="pout")
            for g in range(NG):
                nc.tensor.transpose(
                    pout[:G, g * F : (g + 1) * F],
                    o3[:, g * G : (g + 1) * G],
                    ident[:, :],
                )

            # --- row L2 norm ---
            sq = npool.tile([128, NG * F], f32, tag="sq")
            nsq = npool.tile([128, NG], f32, tag="nsq")
            for g in range(NG):
                nc.scalar.activation(
                    sq[:G, g * F : (g + 1) * F],
                    pout[:G, g * F : (g + 1) * F],
                    ACTF.Square,
                    accum_out=nsq[:G, g : g + 1],
                )
            nv = npool.tile([128, NG], f32, tag="nv")
            nc.scalar.activation(nv[:G], nsq[:G], ACTF.Sqrt)
            nve = npool.tile([128, NG], f32, tag="nve")
            nc.vector.tensor_scalar_add(nve[:G], nv[:G], 1e-8)
            ri = npool.tile([128, NG], f32, tag="ri")
            nc.vector.reciprocal(ri[:G], nve[:G])

            outsb = opool.tile([128, NG * F], f32, tag="outsb")
            for g in range(NG):
                nc.vector.tensor_scalar_mul(
                    outsb[:G, g * F : (g + 1) * F],
                    pout[:G, g * F : (g + 1) * F],
                    ri[:G, g : g + 1],
                )

            # --- store ---
            nc.scalar.dma_start(
                out_d[r0 : r0 + MACRO].rearrange("(g p) f -> p g f", p=G),
                outsb[:G].rearrange("p (g f) -> p g f", g=NG),
            )

    nc.compile()
    return nc


def _get_nc(n_nodes, n_cores):
    key = (n_nodes, n_cores)
    if key not in _NC_CACHE:
        _NC_CACHE[key] = build(n_nodes, n_cores)
    return _NC_CACHE[key]


def kernel(message, features, global_features, W1, b1, W2, b2, W3, b3):
    from concourse.bass_utils import run_bass_kernel_spmd

    n = message.shape[0]
    assert n % N_CORES == 0
    npc = n // N_CORES

    nc = _get_nc(npc, N_CORES)

    def shard(a, shape):
        return np.ascontiguousarray(
            np.asarray(a, dtype=np.float32).reshape((N_CORES,) + shape)
        )

    msg = shard(message, (npc, DEG * F))
    feat = shard(features, (npc, F))
    glob = shard(global_features, (npc, F))
    w1 = np.ascontiguousarray(np.asarray(W1, np.float32))
    w2 = np.ascontiguousarray(np.asarray(W2, np.float32))
    w3 = np.ascontiguousarray(np.asarray(W3, np.float32))
    bb1 = np.ascontiguousarray(np.asarray(b1, np.float32))
    bb2 = np.ascontiguousarray(np.asarray(b2, np.float32))
    bb3 = np.ascontiguousarray(np.asarray(b3, np.float32))

    in_maps = [
        {
            "message": msg[i],
            "features": feat[i],
            "global_features": glob[i],
            "W1": w1,
            "b1": bb1,
            "W2": w2,
            "b2": bb2,
            "W3": w3,
            "b3": bb3,
        }
        for i in range(N_CORES)
    ]
    res = run_bass_kernel_spmd(nc, in_maps, list(range(N_CORES))).results
    return np.concatenate([res[i]["out"] for i in range(N_CORES)], axis=0)

